# revision 57
# baseline (speedup 1.0000x reference)
"""Causal attention block (B=2, S=2048, H=1024, 16 heads) on 8 NeuronCores.

Sharding: core c handles batch b = c // 4 and head-group g = c % 4
(4 heads = 256 qkv columns / w_out rows per core). Each core computes a
partial output y_partial = softmax(QK^T/sqrt(d)) V @ Wout_slice for its
heads; the host sums the 4 head-group partials per batch.

Design (bf16 compute, f32 PSUM):
  x^T [1024, 2048] bf16 per 512-col s-chunk; Q^T,K^T head-pair tiles
  [128, 2048] bf16 (d on partitions); V natural [t, d] per t-chunk in
  VA [128, 260] bf16 with an interleaved ones column per head (cols
  65h+64), so the PV matmul emits softmax denominators for free.
  Scores: S^T = K^T.T @ Q^T per (head-pair, t-chunk, s-chunk) into a
  2-bank psum pair tile; ONE exp per pair (2-region AP) -> persistent
  pt [128, 1024] bf16; diagonal triangles zeroed post-exp by a single
  gpsimd affine_select over both heads (pattern [[0,2],[1,128]]).
  PV flipped to [s, d] orientation: the pt block is the *stationary*
  operand and VA the 65-wide moving operand, so each PV step costs 65
  rows instead of 512. PSUM allows only ONE open accumulation group per
  bank, so each (s-tile, head) accumulates over its whole t-range as one
  unbroken matmul group, batched at diagonal steps (pt tiles persist per
  chunk). Z lands per-partition -> reciprocal + per-partition scalar
  multiply (no broadcast dance), then one PE transpose (identity rhs)
  per head pair into VT [d, s] for the out-projection. Transposes write
  into fresh 2-bank slots only: a transpose clobbers its whole psum bank.
  Schedule: software pipeline with PV batches lagging S/exp one step and
  transposes two; next-chunk QKV chains are deadline-scheduled PE filler;
  all out-projections (half-H sub-units) are deferred into the ACT-bound
  final chunk, with tail ysb copies alternating DVE/ACT.
  PSUM: 3 x 2-bank score slots + 2 rotating banks (PV groups / chains /
  out-proj / transposes).  Queues: x^T + y on sync(SP), weights on
  scalar(ACT) issued before any exp.
"""

import numpy as np
from collections import deque
from contextlib import ExitStack

import concourse.bass as bass
import concourse.tile as tile
import concourse.mybir as mybir
from concourse import bacc
from concourse import bass_utils

F32 = mybir.dt.float32
BF16 = mybir.dt.bfloat16
AF = mybir.ActivationFunctionType

B, S, H = 2, 2048, 1024
NH, DH = 16, 64
NCORES = 8
SC = 512            # s-chunk width
NSC = S // SC       # 4
NHC = H // 128      # 8 h contraction chunks

_CACHE = {}


def _gstep(j):
    return 2 * j * (j + 1)


def _build():
    nc = bacc.Bacc("TRN2", target_bir_lowering=False, debug=False,
                   enable_asserts=False, num_devices=NCORES)
    xT = nc.dram_tensor("xT", [H, S], BF16, kind="ExternalInput").ap()
    wq = nc.dram_tensor("wq", [H, 256], BF16, kind="ExternalInput").ap()
    wk = nc.dram_tensor("wk", [H, 256], BF16, kind="ExternalInput").ap()
    wv = nc.dram_tensor("wv", [H, 256], BF16, kind="ExternalInput").ap()
    wo = nc.dram_tensor("wo", [256, H], BF16, kind="ExternalInput").ap()
    aux = nc.dram_tensor("aux", [128, 132], BF16, kind="ExternalInput").ap()
    y = nc.dram_tensor("y", [S, H], BF16, kind="ExternalOutput").ap()
    import os
    KDBG = os.environ.get("KDBG", "0") == "1"
    if KDBG:
        dbg = {nm: nc.dram_tensor(nm, shp, dt, kind="ExternalOutput").ap()
               for nm, shp, dt in [
                   ("d_vt0", [128, S], BF16), ("d_vt1", [128, S], BF16),
                   ("d_qt0", [128, S], BF16), ("d_kt0", [128, S], BF16),
                   ("d_va0", [128, 256], BF16), ("d_zz0", [128, 16], F32),
                   ("d_v2_0", [128, 256], BF16), ("d_pt00", [128, SC], BF16),
                   ("d_pt01", [128, SC], BF16), ("d_pv1", [128, 64], F32),
                   ("d_v2_1", [128, 256], BF16),
               ]}

    with tile.TileContext(nc) as tc:
        with ExitStack() as ctx:
            pw = ctx.enter_context(tc.tile_pool(name="w", bufs=1))
            pxt = ctx.enter_context(tc.tile_pool(name="xt", bufs=2))
            pbig = ctx.enter_context(tc.tile_pool(name="big", bufs=1))
            ppt = ctx.enter_context(tc.tile_pool(name="pt", bufs=36))
            pzz = ctx.enter_context(tc.tile_pool(name="zz", bufs=2))
            pv2 = ctx.enter_context(tc.tile_pool(name="v2", bufs=3))
            pyo = ctx.enter_context(tc.tile_pool(name="yo", bufs=4))
            psum = ctx.enter_context(
                tc.tile_pool(name="psum", bufs=1, space="PSUM"))

            def s_tile(name):
                # 2-bank slots shared by score-pairs, chains, out-proj, tr
                return psum.tile([128, 2 * SC], F32, tag="s", bufs=3,
                                 name=name)

            # ---- weights on the scalar (ACT) DGE queue, before any exp ----
            def load_w_all(dram, nm, splits=(8,)):
                t = pw.tile([128, NHC * 256], BF16, tag=nm, name=nm)
                dst = t[:].rearrange("p (c n) -> p c n", c=NHC)
                src = dram.rearrange("(c p) n -> p c n", p=128)
                lo = 0
                for hi in splits:
                    nc.scalar.dma_start(dst[:, lo:hi, :], src[:, lo:hi, :])
                    lo = hi
                return [t[:, hc * 256:(hc + 1) * 256] for hc in range(NHC)]

            wq_t = load_w_all(wq, "wqa", splits=(4, 8))
            wk_t = load_w_all(wk, "wka")
            aux_t = pw.tile([128, 132], BF16, tag="aux")
            nc.scalar.dma_start(aux_t[:], aux[:])
            ident = aux_t[:, 0:128]
            ones1 = aux_t[:, 128:129]
            wv_t = load_w_all(wv, "wva")
            wo_t = []
            for p in range(2):
                t = pw.tile([128, H], BF16, tag=f"wo{p}", name=f"wo{p}")
                nc.scalar.dma_start(t[:], wo[p * 128:(p + 1) * 128, :])
                wo_t.append(t)

            # ---- persistent activations ----
            QT = [pbig.tile([128, S], BF16, tag=f"qt{p}", name=f"qt{p}")
                  for p in range(2)]
            KT = [pbig.tile([128, S], BF16, tag=f"kt{p}", name=f"kt{p}")
                  for p in range(2)]
            VT = [pbig.tile([128, S], BF16, tag=f"vt{p}", name=f"vt{p}")
                  for p in range(2)]
            VA = [pbig.tile([128, 260], BF16, tag=f"va{t_}", name=f"va{t_}")
                  for t_ in range(S // 128)]
            for t_ in range(S // 128):
                ones_cols = VA[t_][:].rearrange(
                    "q (h c) -> q h c", c=65)[:, :, 64]
                nc.gpsimd.memset(ones_cols, 1.0)

            xt_tiles = [None] * NSC

            def load_xt(j, splits):
                xt_all = pxt.tile([128, NHC * SC], BF16, tag="xt",
                                  name=f"xt{j}")
                sj = slice(j * SC, (j + 1) * SC)
                xt_src = xT.rearrange("(c p) s -> p c s", p=128)[:, :, sj]
                xt_dst = xt_all[:].rearrange("p (c s) -> p c s", c=NHC)
                lo = 0
                for hi in splits:
                    nc.sync.dma_start(xt_dst[:, lo:hi, :],
                                      xt_src[:, lo:hi, :])
                    lo = hi
                xt_tiles[j] = xt_all

            # ---- QKV projection chains ----
            on_diag = [False]

            def qk_chain(j, W, OUT, p, tag=None):
                def emit():
                    xt_all = xt_tiles[j]
                    sj = slice(j * SC, (j + 1) * SC)
                    t_ = tag or ("s" if on_diag[0] else "pv")
                    if t_ == "s":
                        ps = s_tile(f"qk{j}_{p}")[:, 0:SC]
                    else:
                        ps = psum.tile([128, SC], F32, tag="pv", bufs=2,
                                       name=f"qk{j}_{p}")
                    for hc in range(NHC):
                        nc.tensor.matmul(
                            ps[:], W[hc][:, p * 128:(p + 1) * 128],
                            xt_all[:, hc * SC:(hc + 1) * SC],
                            start=(hc == 0), stop=(hc == NHC - 1))
                    nc.vector.tensor_copy(OUT[p][:, sj], ps[:])
                return emit

            def v_chain(j, tci):
                def emit():
                    xt_all = xt_tiles[j]
                    t_ = 4 * j + tci
                    if on_diag[0]:
                        ps = s_tile(f"v{j}_{tci}")[:, 0:SC]
                    else:
                        ps = psum.tile([128, SC], F32, tag="pv", bufs=2,
                                       name=f"v{j}_{tci}")
                    for hc in range(NHC):
                        nc.tensor.matmul(
                            ps[:, 0:256],
                            xt_all[:, hc * SC + tci * 128:
                                   hc * SC + (tci + 1) * 128],
                            wv_t[hc], start=(hc == 0), stop=(hc == NHC - 1))
                    dst = VA[t_][:].rearrange(
                        "q (h c) -> q h c", c=65)[:, :, 0:64]
                    nc.vector.tensor_copy(
                        dst, ps[:, 0:256].rearrange(
                            "q (h c) -> q h c", c=64))
                return emit

            # ---- chunk-local state: pt tiles persist per chunk ----
            class ChunkCtx:
                def __init__(self, j):
                    self.pts = {}   # (tcc, h) -> pt tile
                    self.zz = pzz.tile([128, 16], F32, tag="zz",
                                       name=f"zz{j}")

            # ---- attention pieces ----
            def emit_S(cc, j, tcc, prs):
                k = tcc - 4 * j
                c0 = max(0, 128 * k)
                sjv = slice(j * SC + c0, (j + 1) * SC)
                for p in prs:
                    ss = s_tile(f"ss{tcc}_{p}")
                    for r in range(2):
                        nc.tensor.matmul(
                            ss[:, SC * r + c0:SC * (r + 1)],
                            KT[p][64 * r:64 * (r + 1),
                                  tcc * 128:(tcc + 1) * 128],
                            QT[p][64 * r:64 * (r + 1), sjv],
                            start=True, stop=True)
                    pt = ppt.tile([128, 2 * SC], BF16, tag="pt")
                    w2 = SC - c0
                    src2 = ss[:].rearrange("q (r s) -> q r s", r=2)[
                        :, :, c0:SC]
                    dst2 = pt[:].rearrange("q (r s) -> q r s", r=2)[
                        :, :, c0:SC]
                    nc.scalar.activation(dst2, src2, AF.Exp)
                    if k >= 0:
                        band = pt[:].rearrange("q (r s) -> q r s", r=2)[
                            :, :, c0:c0 + 128]
                        nc.gpsimd.affine_select(
                            band, band,
                            pattern=[[0, 2], [1, 128]], base=0,
                            channel_multiplier=-1,
                            compare_op=mybir.AluOpType.is_ge, fill=0.0)
                    cc.pts[(tcc, p)] = pt

            subs = deque()        # deferred out-projection sub-closures
            pe_extras = deque()   # deferred transpose closures

            def make_transpose(cc, j, sti, v2):
                st = 4 * j + sti

                def emit():
                    for p in range(2):
                        trt = s_tile(f"tr{st}_{p}")[:, 0:SC]
                        trp = trt[:, 0:64].bitcast(BF16)
                        nc.tensor.transpose(
                            trp, v2[:, 128 * p:128 * (p + 1)], ident)
                        nc.vector.tensor_copy(
                            VT[p][:, st * 128:(st + 1) * 128], trp)
                    ysb = pyo.tile([128, H], BF16, tag="y", name=f"ysb{st}")
                    for n2 in range(2):
                        subs.append(make_sub(st, n2, ysb))
                return emit

            def make_sub(st, n2, ysb):
                def emit():
                    py_ = psum.tile([128, SC], F32, tag="pv", bufs=2,
                                    name=f"py{st}_{n2}")
                    for p in range(2):
                        nc.tensor.matmul(
                            py_[:], VT[p][:, st * 128:(st + 1) * 128],
                            wo_t[p][:, n2 * 512:(n2 + 1) * 512],
                            start=(p == 0), stop=(p == 1))
                    if st >= 12 and n2 == 1:
                        nc.scalar.copy(
                            ysb[:, n2 * 512:(n2 + 1) * 512], py_[:])
                    else:
                        nc.vector.tensor_copy(
                            ysb[:, n2 * 512:(n2 + 1) * 512], py_[:])
                    nc.sync.dma_start(
                        y[st * 128:(st + 1) * 128, n2 * 512:(n2 + 1) * 512],
                        ysb[:, n2 * 512:(n2 + 1) * 512])
                return emit

            def emit_PV(cc, j, sti):
                # one unbroken accumulation group per (s-tile, head):
                # PV over tcc=0..st, then z over tcc=0..st, sequentially
                # through one psum bank (one open group per bank at a time)
                st = 4 * j + sti
                bank = psum.tile([128, SC], F32, tag="pv", bufs=2,
                                 name=f"pv{st}")
                for h in range(4):
                    p_, r_ = divmod(h, 2)
                    o_ = SC * r_ + sti * 128
                    for tcc in range(st + 1):
                        ptsl = cc.pts[(tcc, p_)][:, o_:o_ + 128]
                        nc.tensor.matmul(
                            bank[:, 65 * h:65 * (h + 1)], ptsl,
                            VA[tcc][:, 65 * h:65 * (h + 1)],
                            start=(tcc == 0), stop=(tcc == st))
                nc.vector.reciprocal(
                    cc.zz[:, 4 * sti:4 * sti + 4],
                    bank[:, 0:260].rearrange(
                        "q (h c) -> q h c", c=65)[:, :, 64])
                v2 = pv2.tile([128, 256], BF16, tag="v2", name=f"v2_{st}")
                for h in range(4):
                    nc.vector.tensor_scalar_mul(
                        v2[:, 64 * h:64 * (h + 1)],
                        bank[:, 65 * h:65 * h + 64],
                        cc.zz[:, 4 * sti + h:4 * sti + h + 1])
                if KDBG and st == 0:
                    nc.sync.dma_start(dbg["d_v2_0"][:], v2[:])
                if KDBG and st == 1:
                    nc.sync.dma_start(dbg["d_zz0"][:], cc.zz[:])
                    nc.sync.dma_start(dbg["d_v2_1"][:], v2[:])
                    dsb = pw.tile([128, 64], F32, tag="dsb")
                    nc.vector.tensor_copy(dsb[:], bank[:, 0:64])
                    nc.sync.dma_start(dbg["d_pv1"][:], dsb[:])
                pe_extras.append(make_transpose(cc, j, sti, v2))

            # ---- global schedule ----
            chains = deque()   # (deadline_step, emit_fn)

            load_xt(0, (2, 4, 6, 8))
            qk_chain(0, wq_t, QT, 0, tag="pv")()
            qk_chain(0, wk_t, KT, 0, tag="s")()
            qk_chain(0, wq_t, QT, 1, tag="s")()
            qk_chain(0, wk_t, KT, 1, tag="s")()
            for tci in range(4):
                chains.append((tci + 1, v_chain(0, tci)))

            pending = None
            g = 0
            for j in range(NSC):
                ntc = 4 * j + 4
                cc = ChunkCtx(j)
                if j + 1 < NSC:
                    load_xt(j + 1, (4, 8))
                    g1 = _gstep(j + 1)
                    for p in range(2):
                        chains.append((g1, qk_chain(j + 1, wq_t, QT, p)))
                    for p in range(2):
                        chains.append((g1 + 4 * (j + 1),
                                       qk_chain(j + 1, wk_t, KT, p)))
                    for tci in range(4):
                        chains.append((g1 + 4 * (j + 1) + tci,
                                       v_chain(j + 1, tci)))
                for tcc in range(ntc):
                    k = tcc - 4 * j
                    on_diag[0] = k >= 0
                    emit_S(cc, j, tcc, [0])
                    while pe_extras:
                        pe_extras.popleft()()
                    if pending is not None:
                        emit_PV(*pending)
                        pending = None
                    emit_S(cc, j, tcc, [1])
                    if k >= 0:
                        pending = (cc, j, k)
                    # non-S psum users this step: <= 1 chain + <= 1 sub
                    popped = 0
                    while chains and (chains[0][0] <= g + 1
                                      or (popped == 0
                                          and chains[0][0] <= g + 5)):
                        chains.popleft()[1]()
                        popped += 1
                        if popped >= 2 and not (chains
                                                and chains[0][0] <= g + 1):
                            break
                    nsub = (2 if (j == 3 and k < 0) else
                            (1 if (k < 0 and ((j == 2 and tcc >= 2
                                               and len(subs) > 8)
                                              or (j == 1 and tcc >= 2
                                                  and len(subs) > 4)))
                             else 0))
                    for _ in range(min(nsub, len(subs))):
                        subs.popleft()()
                    g += 1
            emit_PV(*pending)
            while pe_extras:
                pe_extras.popleft()()
            while subs:
                subs.popleft()()
            if KDBG:
                nc.sync.dma_start(dbg["d_vt0"][:], VT[0][:])
                nc.sync.dma_start(dbg["d_vt1"][:], VT[1][:])
                nc.sync.dma_start(dbg["d_qt0"][:], QT[0][:])
                nc.sync.dma_start(dbg["d_kt0"][:], KT[0][:])
                nc.sync.dma_start(dbg["d_va0"][:], VA[0][:])
    nc.compile()
    return nc


def _in_maps(x, w_qkv, w_out):
    import ml_dtypes
    bf16 = ml_dtypes.bfloat16
    x = np.asarray(x, dtype=np.float32)
    w_qkv = np.asarray(w_qkv, dtype=np.float32)
    w_out = np.asarray(w_out, dtype=np.float32)
    aux_const = np.zeros((128, 132), dtype=np.float32)
    aux_const[:, 0:128] = np.eye(128, dtype=np.float32)
    aux_const[:, 128] = 1.0
    aux_const = aux_const.astype(bf16)
    scale = np.float32(1.0 / np.sqrt(DH))
    in_maps = []
    for c in range(NCORES):
        b, g = divmod(c, 4)
        cols = slice(256 * g, 256 * (g + 1))
        in_maps.append({
            "xT": np.ascontiguousarray(x[b].T).astype(bf16),
            "wq": (np.ascontiguousarray(w_qkv[:, 0 * H:1 * H][:, cols])
                   * scale).astype(bf16),
            "wk": np.ascontiguousarray(
                w_qkv[:, 1 * H:2 * H][:, cols]).astype(bf16),
            "wv": np.ascontiguousarray(
                w_qkv[:, 2 * H:3 * H][:, cols]).astype(bf16),
            "wo": np.ascontiguousarray(w_out[cols, :]).astype(bf16),
            "aux": aux_const,
        })
    return in_maps


TRACE = False
LAST_RESULTS = None


def kernel(x, w_qkv, w_out):
    global LAST_RESULTS
    if "nc" not in _CACHE:
        _CACHE["nc"] = _build()
    nc = _CACHE["nc"]
    in_maps = _in_maps(x, w_qkv, w_out)
    res = bass_utils.run_bass_kernel_spmd(
        nc, in_maps, core_ids=list(range(NCORES)), trace=TRACE)
    LAST_RESULTS = res
    y = np.zeros((B, S, H), dtype=np.float32)
    for c in range(NCORES):
        y[c // 4] += np.asarray(res.results[c]["y"], dtype=np.float32)
    return y


# revision 64
# speedup vs baseline: 1.0017x; 1.0017x over previous
"""Causal attention block (B=2, S=2048, H=1024, 16 heads) on 8 NeuronCores.

Sharding: core c handles batch b = c // 4 and head-group g = c % 4
(4 heads = 256 qkv columns / w_out rows per core). Each core computes a
partial output y_partial = softmax(QK^T/sqrt(d)) V @ Wout_slice for its
heads; the host sums the 4 head-group partials per batch.

Design (bf16 compute, f32 PSUM):
  x^T [1024, 2048] bf16 per 512-col s-chunk; Q^T,K^T head-pair tiles
  [128, 2048] bf16 (d on partitions); V natural [t, d] per t-chunk in
  VA [128, 260] bf16 with an interleaved ones column per head (cols
  65h+64), so the PV matmul emits softmax denominators for free.
  Scores: S^T = K^T.T @ Q^T per (head-pair, t-chunk, s-chunk) into a
  2-bank psum pair tile; ONE exp per pair (2-region AP) -> persistent
  pt [128, 1024] bf16; diagonal triangles zeroed post-exp by a single
  gpsimd affine_select over both heads (pattern [[0,2],[1,128]]).
  PV flipped to [s, d] orientation: the pt block is the *stationary*
  operand and VA the 65-wide moving operand, so each PV step costs 65
  rows instead of 512. PSUM allows only ONE open accumulation group per
  bank, so each (s-tile, head) accumulates over its whole t-range as one
  unbroken matmul group, batched at diagonal steps (pt tiles persist per
  chunk). Z lands per-partition -> reciprocal + per-partition scalar
  multiply (no broadcast dance), then one PE transpose (identity rhs)
  per head pair into VT [d, s] for the out-projection. Transposes write
  into fresh 2-bank slots only: a transpose clobbers its whole psum bank.
  Schedule: software pipeline with PV batches lagging S/exp one step and
  transposes two; next-chunk QKV chains are deadline-scheduled PE filler;
  all out-projections (half-H sub-units) are deferred into the ACT-bound
  final chunk, with tail ysb copies alternating DVE/ACT.
  PSUM: 3 x 2-bank score slots + 2 rotating banks (PV groups / chains /
  out-proj / transposes).  Queues: x^T + y on sync(SP), weights on
  scalar(ACT) issued before any exp.
"""

import numpy as np
from collections import deque
from contextlib import ExitStack

import concourse.bass as bass
import concourse.tile as tile
import concourse.mybir as mybir
from concourse import bacc
from concourse import bass_utils

F32 = mybir.dt.float32
BF16 = mybir.dt.bfloat16
AF = mybir.ActivationFunctionType

B, S, H = 2, 2048, 1024
NH, DH = 16, 64
NCORES = 8
SC = 512            # s-chunk width
NSC = S // SC       # 4
NHC = H // 128      # 8 h contraction chunks

_CACHE = {}


def _gstep(j):
    return 2 * j * (j + 1)


def _build():
    nc = bacc.Bacc("TRN2", target_bir_lowering=False, debug=False,
                   enable_asserts=False, num_devices=NCORES)
    xT = nc.dram_tensor("xT", [H, S], BF16, kind="ExternalInput").ap()
    wq = nc.dram_tensor("wq", [H, 256], BF16, kind="ExternalInput").ap()
    wk = nc.dram_tensor("wk", [H, 256], BF16, kind="ExternalInput").ap()
    wv = nc.dram_tensor("wv", [H, 256], BF16, kind="ExternalInput").ap()
    wo = nc.dram_tensor("wo", [256, H], BF16, kind="ExternalInput").ap()
    aux = nc.dram_tensor("aux", [128, 132], BF16, kind="ExternalInput").ap()
    y = nc.dram_tensor("y", [S, H], BF16, kind="ExternalOutput").ap()
    import os
    KDBG = os.environ.get("KDBG", "0") == "1"
    if KDBG:
        dbg = {nm: nc.dram_tensor(nm, shp, dt, kind="ExternalOutput").ap()
               for nm, shp, dt in [
                   ("d_vt0", [128, S], BF16), ("d_vt1", [128, S], BF16),
                   ("d_qt0", [128, S], BF16), ("d_kt0", [128, S], BF16),
                   ("d_va0", [128, 256], BF16), ("d_zz0", [128, 16], F32),
                   ("d_v2_0", [128, 256], BF16), ("d_pt00", [128, SC], BF16),
                   ("d_pt01", [128, SC], BF16), ("d_pv1", [128, 64], F32),
                   ("d_v2_1", [128, 256], BF16),
               ]}

    with tile.TileContext(nc) as tc:
        with ExitStack() as ctx:
            pw = ctx.enter_context(tc.tile_pool(name="w", bufs=1))
            pxt = ctx.enter_context(tc.tile_pool(name="xt", bufs=2))
            pbig = ctx.enter_context(tc.tile_pool(name="big", bufs=1))
            ppt = ctx.enter_context(tc.tile_pool(name="pt", bufs=36))
            pzz = ctx.enter_context(tc.tile_pool(name="zz", bufs=2))
            pv2 = ctx.enter_context(tc.tile_pool(name="v2", bufs=3))
            pyo = ctx.enter_context(tc.tile_pool(name="yo", bufs=4))
            psum = ctx.enter_context(
                tc.tile_pool(name="psum", bufs=1, space="PSUM"))

            def s_tile(name):
                # 2-bank slots shared by score-pairs, chains, out-proj, tr
                return psum.tile([128, 2 * SC], F32, tag="s", bufs=3,
                                 name=name)

            # ---- weights on the scalar (ACT) DGE queue, before any exp ----
            def load_w_all(dram, nm, splits=(8,)):
                t = pw.tile([128, NHC * 256], BF16, tag=nm, name=nm)
                dst = t[:].rearrange("p (c n) -> p c n", c=NHC)
                src = dram.rearrange("(c p) n -> p c n", p=128)
                lo = 0
                for hi in splits:
                    nc.scalar.dma_start(dst[:, lo:hi, :], src[:, lo:hi, :])
                    lo = hi
                return [t[:, hc * 256:(hc + 1) * 256] for hc in range(NHC)]

            wq_t = load_w_all(wq, "wqa", splits=(4, 8))
            wk_t = load_w_all(wk, "wka")
            aux_t = pw.tile([128, 132], BF16, tag="aux")
            nc.scalar.dma_start(aux_t[:], aux[:])
            ident = aux_t[:, 0:128]
            ones1 = aux_t[:, 128:129]
            wv_t = load_w_all(wv, "wva")
            wo_t = []
            for p in range(2):
                t = pw.tile([128, H], BF16, tag=f"wo{p}", name=f"wo{p}")
                nc.scalar.dma_start(t[:], wo[p * 128:(p + 1) * 128, :])
                wo_t.append(t)

            # ---- persistent activations ----
            QT = [pbig.tile([128, S], BF16, tag=f"qt{p}", name=f"qt{p}")
                  for p in range(2)]
            KT = [pbig.tile([128, S], BF16, tag=f"kt{p}", name=f"kt{p}")
                  for p in range(2)]
            VT = [pbig.tile([128, S], BF16, tag=f"vt{p}", name=f"vt{p}")
                  for p in range(2)]
            VA = [pbig.tile([128, 260], BF16, tag=f"va{t_}", name=f"va{t_}")
                  for t_ in range(S // 128)]
            for t_ in range(S // 128):
                ones_cols = VA[t_][:].rearrange(
                    "q (h c) -> q h c", c=65)[:, :, 64]
                nc.gpsimd.memset(ones_cols, 1.0)

            # PE p-state warmup during the DMA-latency head
            wu = pw.tile([128, SC], BF16, tag="wu")
            nc.gpsimd.memset(wu[:], 1.0)
            wups = psum.tile([128, SC], F32, tag="pv", bufs=2, name="wups")
            for wi in range(3):
                nc.tensor.matmul(wups[:], wu[:, 0:128], wu[:],
                                 start=True, stop=True)

            xt_tiles = [None] * NSC

            def load_xt(j, splits):
                xt_all = pxt.tile([128, NHC * SC], BF16, tag="xt",
                                  name=f"xt{j}")
                sj = slice(j * SC, (j + 1) * SC)
                xt_src = xT.rearrange("(c p) s -> p c s", p=128)[:, :, sj]
                xt_dst = xt_all[:].rearrange("p (c s) -> p c s", c=NHC)
                lo = 0
                for hi in splits:
                    nc.sync.dma_start(xt_dst[:, lo:hi, :],
                                      xt_src[:, lo:hi, :])
                    lo = hi
                xt_tiles[j] = xt_all

            # ---- QKV projection chains ----
            on_diag = [False]

            def qk_chain(j, W, OUT, p, tag=None):
                def emit():
                    xt_all = xt_tiles[j]
                    sj = slice(j * SC, (j + 1) * SC)
                    t_ = tag or ("s" if on_diag[0] else "pv")
                    if t_ == "s":
                        ps = s_tile(f"qk{j}_{p}")[:, 0:SC]
                    else:
                        ps = psum.tile([128, SC], F32, tag="pv", bufs=2,
                                       name=f"qk{j}_{p}")
                    for hc in range(NHC):
                        nc.tensor.matmul(
                            ps[:], W[hc][:, p * 128:(p + 1) * 128],
                            xt_all[:, hc * SC:(hc + 1) * SC],
                            start=(hc == 0), stop=(hc == NHC - 1))
                    nc.vector.tensor_copy(OUT[p][:, sj], ps[:])
                return emit

            def v_chain(j, tci):
                def emit():
                    xt_all = xt_tiles[j]
                    t_ = 4 * j + tci
                    if on_diag[0]:
                        ps = s_tile(f"v{j}_{tci}")[:, 0:SC]
                    else:
                        ps = psum.tile([128, SC], F32, tag="pv", bufs=2,
                                       name=f"v{j}_{tci}")
                    for hc in range(NHC):
                        nc.tensor.matmul(
                            ps[:, 0:256],
                            xt_all[:, hc * SC + tci * 128:
                                   hc * SC + (tci + 1) * 128],
                            wv_t[hc], start=(hc == 0), stop=(hc == NHC - 1))
                    dst = VA[t_][:].rearrange(
                        "q (h c) -> q h c", c=65)[:, :, 0:64]
                    nc.vector.tensor_copy(
                        dst, ps[:, 0:256].rearrange(
                            "q (h c) -> q h c", c=64))
                return emit

            # ---- chunk-local state: pt tiles persist per chunk ----
            class ChunkCtx:
                def __init__(self, j):
                    self.pts = {}   # (tcc, h) -> pt tile
                    self.zz = pzz.tile([128, 16], F32, tag="zz",
                                       name=f"zz{j}")

            # ---- attention pieces ----
            def emit_S(cc, j, tcc, prs):
                k = tcc - 4 * j
                c0 = max(0, 128 * k)
                sjv = slice(j * SC + c0, (j + 1) * SC)
                for p in prs:
                    ss = s_tile(f"ss{tcc}_{p}")
                    for r in range(2):
                        nc.tensor.matmul(
                            ss[:, SC * r + c0:SC * (r + 1)],
                            KT[p][64 * r:64 * (r + 1),
                                  tcc * 128:(tcc + 1) * 128],
                            QT[p][64 * r:64 * (r + 1), sjv],
                            start=True, stop=True)
                    pt = ppt.tile([128, 2 * SC], BF16, tag="pt")
                    w2 = SC - c0
                    src2 = ss[:].rearrange("q (r s) -> q r s", r=2)[
                        :, :, c0:SC]
                    dst2 = pt[:].rearrange("q (r s) -> q r s", r=2)[
                        :, :, c0:SC]
                    nc.scalar.activation(dst2, src2, AF.Exp)
                    if k >= 0:
                        band = pt[:].rearrange("q (r s) -> q r s", r=2)[
                            :, :, c0:c0 + 128]
                        nc.gpsimd.affine_select(
                            band, band,
                            pattern=[[0, 2], [1, 128]], base=0,
                            channel_multiplier=-1,
                            compare_op=mybir.AluOpType.is_ge, fill=0.0)
                    cc.pts[(tcc, p)] = pt

            subs = deque()        # deferred out-projection sub-closures
            pe_extras = deque()   # deferred transpose closures

            def make_transpose(cc, j, sti, v2):
                st = 4 * j + sti

                def emit():
                    for p in range(2):
                        trt = s_tile(f"tr{st}_{p}")[:, 0:SC]
                        trp = trt[:, 0:64].bitcast(BF16)
                        nc.tensor.transpose(
                            trp, v2[:, 128 * p:128 * (p + 1)], ident)
                        nc.vector.tensor_copy(
                            VT[p][:, st * 128:(st + 1) * 128], trp)
                    ysb = pyo.tile([128, H], BF16, tag="y", name=f"ysb{st}")
                    for n2 in range(2):
                        subs.append(make_sub(st, n2, ysb))
                return emit

            def make_sub(st, n2, ysb):
                def emit():
                    py_ = psum.tile([128, SC], F32, tag="pv", bufs=2,
                                    name=f"py{st}_{n2}")
                    for p in range(2):
                        nc.tensor.matmul(
                            py_[:], VT[p][:, st * 128:(st + 1) * 128],
                            wo_t[p][:, n2 * 512:(n2 + 1) * 512],
                            start=(p == 0), stop=(p == 1))
                    if st >= 12 and n2 == 1:
                        nc.scalar.copy(
                            ysb[:, n2 * 512:(n2 + 1) * 512], py_[:])
                    else:
                        nc.vector.tensor_copy(
                            ysb[:, n2 * 512:(n2 + 1) * 512], py_[:])
                    nc.sync.dma_start(
                        y[st * 128:(st + 1) * 128, n2 * 512:(n2 + 1) * 512],
                        ysb[:, n2 * 512:(n2 + 1) * 512])
                return emit

            def emit_PV(cc, j, sti):
                # one unbroken accumulation group per (s-tile, head):
                # PV over tcc=0..st, then z over tcc=0..st, sequentially
                # through one psum bank (one open group per bank at a time)
                st = 4 * j + sti
                bank = psum.tile([128, SC], F32, tag="pv", bufs=2,
                                 name=f"pv{st}")
                for h in range(4):
                    p_, r_ = divmod(h, 2)
                    o_ = SC * r_ + sti * 128
                    for tcc in range(st + 1):
                        ptsl = cc.pts[(tcc, p_)][:, o_:o_ + 128]
                        nc.tensor.matmul(
                            bank[:, 65 * h:65 * (h + 1)], ptsl,
                            VA[tcc][:, 65 * h:65 * (h + 1)],
                            start=(tcc == 0), stop=(tcc == st))
                nc.vector.reciprocal(
                    cc.zz[:, 4 * sti:4 * sti + 4],
                    bank[:, 0:260].rearrange(
                        "q (h c) -> q h c", c=65)[:, :, 64])
                v2 = pv2.tile([128, 256], BF16, tag="v2", name=f"v2_{st}")
                for h in range(4):
                    nc.vector.tensor_scalar_mul(
                        v2[:, 64 * h:64 * (h + 1)],
                        bank[:, 65 * h:65 * h + 64],
                        cc.zz[:, 4 * sti + h:4 * sti + h + 1])
                if KDBG and st == 0:
                    nc.sync.dma_start(dbg["d_v2_0"][:], v2[:])
                if KDBG and st == 1:
                    nc.sync.dma_start(dbg["d_zz0"][:], cc.zz[:])
                    nc.sync.dma_start(dbg["d_v2_1"][:], v2[:])
                    dsb = pw.tile([128, 64], F32, tag="dsb")
                    nc.vector.tensor_copy(dsb[:], bank[:, 0:64])
                    nc.sync.dma_start(dbg["d_pv1"][:], dsb[:])
                pe_extras.append(make_transpose(cc, j, sti, v2))

            # ---- global schedule ----
            chains = deque()   # (deadline_step, emit_fn)

            load_xt(0, (2, 4, 6, 8))
            qk_chain(0, wq_t, QT, 0, tag="pv")()
            qk_chain(0, wk_t, KT, 0, tag="s")()
            qk_chain(0, wq_t, QT, 1, tag="s")()
            qk_chain(0, wk_t, KT, 1, tag="s")()
            for tci in range(4):
                chains.append((tci + 1, v_chain(0, tci)))

            pending = None
            g = 0
            for j in range(NSC):
                ntc = 4 * j + 4
                cc = ChunkCtx(j)
                if j + 1 < NSC:
                    load_xt(j + 1, (4, 8))
                    g1 = _gstep(j + 1)
                    for p in range(2):
                        chains.append((g1, qk_chain(j + 1, wq_t, QT, p)))
                    for p in range(2):
                        chains.append((g1 + 4 * (j + 1),
                                       qk_chain(j + 1, wk_t, KT, p)))
                    for tci in range(4):
                        chains.append((g1 + 4 * (j + 1) + tci,
                                       v_chain(j + 1, tci)))
                for tcc in range(ntc):
                    k = tcc - 4 * j
                    on_diag[0] = k >= 0
                    emit_S(cc, j, tcc, [0])
                    while pe_extras:
                        pe_extras.popleft()()
                    if pending is not None:
                        emit_PV(*pending)
                        pending = None
                    emit_S(cc, j, tcc, [1])
                    if k >= 0:
                        pending = (cc, j, k)
                    # non-S psum users this step: <= 1 chain + <= 1 sub
                    popped = 0
                    while chains and (chains[0][0] <= g + 1
                                      or (popped == 0
                                          and chains[0][0] <= g + 5)):
                        chains.popleft()[1]()
                        popped += 1
                        if popped >= 2 and not (chains
                                                and chains[0][0] <= g + 1):
                            break
                    nsub = (2 if (j == 3 and k < 0) else
                            (1 if (k < 0 and ((j == 2 and tcc >= 2
                                               and len(subs) > 8)
                                              or (j == 1 and tcc >= 2
                                                  and len(subs) > 4)))
                             else 0))
                    for _ in range(min(nsub, len(subs))):
                        subs.popleft()()
                    g += 1
            emit_PV(*pending)
            while pe_extras:
                pe_extras.popleft()()
            while subs:
                subs.popleft()()
            if KDBG:
                nc.sync.dma_start(dbg["d_vt0"][:], VT[0][:])
                nc.sync.dma_start(dbg["d_vt1"][:], VT[1][:])
                nc.sync.dma_start(dbg["d_qt0"][:], QT[0][:])
                nc.sync.dma_start(dbg["d_kt0"][:], KT[0][:])
                nc.sync.dma_start(dbg["d_va0"][:], VA[0][:])
    nc.compile()
    return nc


def _in_maps(x, w_qkv, w_out):
    import ml_dtypes
    bf16 = ml_dtypes.bfloat16
    x = np.asarray(x, dtype=np.float32)
    w_qkv = np.asarray(w_qkv, dtype=np.float32)
    w_out = np.asarray(w_out, dtype=np.float32)
    aux_const = np.zeros((128, 132), dtype=np.float32)
    aux_const[:, 0:128] = np.eye(128, dtype=np.float32)
    aux_const[:, 128] = 1.0
    aux_const = aux_const.astype(bf16)
    scale = np.float32(1.0 / np.sqrt(DH))
    in_maps = []
    for c in range(NCORES):
        b, g = divmod(c, 4)
        cols = slice(256 * g, 256 * (g + 1))
        in_maps.append({
            "xT": np.ascontiguousarray(x[b].T).astype(bf16),
            "wq": (np.ascontiguousarray(w_qkv[:, 0 * H:1 * H][:, cols])
                   * scale).astype(bf16),
            "wk": np.ascontiguousarray(
                w_qkv[:, 1 * H:2 * H][:, cols]).astype(bf16),
            "wv": np.ascontiguousarray(
                w_qkv[:, 2 * H:3 * H][:, cols]).astype(bf16),
            "wo": np.ascontiguousarray(w_out[cols, :]).astype(bf16),
            "aux": aux_const,
        })
    return in_maps


TRACE = False
LAST_RESULTS = None


def kernel(x, w_qkv, w_out):
    global LAST_RESULTS
    if "nc" not in _CACHE:
        _CACHE["nc"] = _build()
    nc = _CACHE["nc"]
    in_maps = _in_maps(x, w_qkv, w_out)
    res = bass_utils.run_bass_kernel_spmd(
        nc, in_maps, core_ids=list(range(NCORES)), trace=TRACE)
    LAST_RESULTS = res
    y = np.zeros((B, S, H), dtype=np.float32)
    for c in range(NCORES):
        y[c // 4] += np.asarray(res.results[c]["y"], dtype=np.float32)
    return y
